# revision 1
# baseline (speedup 1.0000x reference)
"""BlockTucker fusion kernel for 8 Trainium2 NeuronCores.

Reference computation (per batch row b):
    h0 = x0 @ W0 + b0; h1 = x1 @ W1 + b1              # [B, 1600]
    per chunk c (20 chunks of 80):
        z[c,o] = sum_{s,t} h0c[s] Wb[c,o,s,t] h1c[t] + bb[c,o]
        z = signsqrt(z); z /= max(||z||_2, 1e-12)
    out = concat(z) @ Wout + bout                      # [B, 3000]

Strategy: pure data parallel over batch (1024 rows/core), bf16 compute.
The bilinear form is an outer-product matmul: per chunk,
P^T[(s,t), b] = h0[s,b]*h1[t,b] is built feature-major and
z^T[o,b] = sum_{st} WbT[(s,t),o] P^T[(s,t),b] accumulates in PSUM.

v2 data-movement design (HW ablation showed the v1 kernel was bound on
SBUF->SBUF replication DMAs that hammer 1-2 of the 16 SBUF AXI ports):
  - both h0 and h1 are evacuated to flat DRAM mirrors; the replicated
    feature-major tiles (h0 row s on 16 partitions / h1 row t on 8) are
    built by ONE batched DRAM->SBUF broadcast DMA each per chunk.
  - Wb is host-repacked so each chunk's 50 k-tiles load as a single
    contiguous 8KB-per-partition DMA.
  - z is staged through DRAM instead of a resident SBUF tile, removing
    the per-chunk SBUF->SBUF repack DMAs.
  - h1 projection runs before h0 so the bilinear feed pipeline starts
    during the h0 projection.
"""

import sys

sys.path.insert(0, "/opt/trn_rl_repo")

from contextlib import ExitStack

import numpy as np
import ml_dtypes

import concourse.bass as bass
import concourse.mybir as mybir
import concourse.tile as tile
from concourse import bacc
from concourse.bass_utils import run_bass_kernel_spmd

BF16 = mybir.dt.bfloat16
F32 = mybir.dt.float32
AF = mybir.ActivationFunctionType

B = 8192
D_IN = 2048
MM = 1600
CHUNKS = 20
CS = 80
D_OUT = 3000
N_CORES = 8
BL = B // N_CORES  # 1024 batch rows per core

K_IN = D_IN // 128  # 16 k-tiles for projections
MT_H = 13  # m-tiles for H (1600 -> 12x128 + 64)
MM_PAD = MT_H * 128  # 1664
KT_BIL = 50  # k-tiles per chunk for bilinear (6400/128)
MT_O = 24  # m-tiles for out (3000 -> 23x128 + 56)
D_OUT_PAD = MT_O * 128  # 3072
NH = BL // 512  # 2 free-dim halves of 512
HALF = 4  # norm-batch granularity (chunks)

CFG = dict(allsync=True, zres=True)


def _h_row_segments(mm0, nrows):
    segs = []
    a = mm0
    while a < mm0 + nrows:
        p0 = a % 128
        kt = a // 128
        n = min(128 - p0, mm0 + nrows - a)
        segs.append((a - mm0, p0, kt, n))
        a += n
    return segs


def build_program(reps=1):
    nc = bacc.Bacc("TRN2", target_bir_lowering=False, debug=False)

    x0T = nc.dram_tensor("x0T", [D_IN, BL], BF16, kind="ExternalInput").ap()
    x1T = nc.dram_tensor("x1T", [D_IN, BL], BF16, kind="ExternalInput").ap()
    w0 = nc.dram_tensor("w0", [128, MT_H, K_IN * 128], BF16, kind="ExternalInput").ap()
    w1 = nc.dram_tensor("w1", [128, MT_H, K_IN * 128], BF16, kind="ExternalInput").ap()
    wbp = nc.dram_tensor(
        "wbp", [CHUNKS, 128, KT_BIL, CS], BF16, kind="ExternalInput"
    ).ap()
    wout = nc.dram_tensor("wout", [128, MT_O, MT_H * 128], BF16, kind="ExternalInput").ap()
    b0c = nc.dram_tensor("b0c", [128, MT_H], F32, kind="ExternalInput").ap()
    b1c = nc.dram_tensor("b1c", [128, MT_H], F32, kind="ExternalInput").ap()
    bbT = nc.dram_tensor("bbT", [CS, CHUNKS], F32, kind="ExternalInput").ap()
    boutc = nc.dram_tensor("boutc", [128, MT_O], F32, kind="ExternalInput").ap()
    outT = nc.dram_tensor("outT", [D_OUT, BL], F32, kind="ExternalOutput").ap()
    h0_dram = nc.dram_tensor("h0_dram", [MM_PAD, BL], BF16).ap()
    h1_dram = nc.dram_tensor("h1_dram", [MM_PAD, BL], BF16).ap()
    z_dram = nc.dram_tensor("z_dram", [MM_PAD, BL], BF16).ap()
    rn_dram = nc.dram_tensor("rn_dram", [CHUNKS, BL], BF16).ap()

    with tile.TileContext(nc) as tc:
        for _ in range(reps):
            _emit(
                tc, nc, x0T, x1T, w0, w1, wbp, wout, b0c, b1c, bbT, boutc,
                outT, h0_dram, h1_dram, z_dram, rn_dram,
            )
    nc.compile()
    return nc


def _ring2(nc):
    return nc.sync if CFG["allsync"] else nc.scalar


def _emit(
    tc, nc, x0T, x1T, w0, w1, wbp, wout, b0c, b1c, bbT, boutc, outT,
    h0_dram, h1_dram, z_dram, rn_dram,
):
    ctx = ExitStack()
    with ctx:
        singles = ctx.enter_context(tc.tile_pool(name="singles", bufs=1))
        mm_psum = ctx.enter_context(tc.tile_pool(name="mm_psum", bufs=2, space="PSUM"))
        zpsum_pool = ctx.enter_context(tc.tile_pool(name="zpsum", bufs=2, space="PSUM"))
        nsq_psum = ctx.enter_context(tc.tile_pool(name="nsq_psum", bufs=1, space="PSUM"))

        # constants / biases
        b0s = singles.tile([128, MT_H], F32)
        nc.sync.dma_start(out=b0s, in_=b0c)
        b1s = singles.tile([128, MT_H], F32)
        nc.sync.dma_start(out=b1s, in_=b1c)
        bbs = singles.tile([CS, CHUNKS], F32)
        nc.sync.dma_start(out=bbs, in_=bbT)
        bouts = singles.tile([128, MT_O], F32)
        nc.sync.dma_start(out=bouts, in_=boutc)
        ones80 = singles.tile([CS, 1], BF16)
        nc.vector.memset(ones80, 1.0)
        # zero the z_dram pad rows so the final matmul's last k-tile can't
        # see stale NaNs (wout pad rows are zero, but 0*NaN=NaN)
        if CFG["zres"]:
            zbig = singles.tile([128, MT_H, BL], BF16, tag="zbig")
            nc.vector.memset(zbig[64:128, MT_H - 1, :], 0.0)
        else:
            zbig = None
            zpad = singles.tile([64, BL], BF16)
            nc.vector.memset(zpad, 0.0)
            nc.sync.dma_start(out=z_dram[MM : MM_PAD, :], in_=zpad)

        nsq_big = singles.tile([CHUNKS, BL], F32)
        nc.vector.memset(nsq_big, 1.0)
        rn_big = singles.tile([CHUNKS, BL], F32)
        rn_bf16 = singles.tile([CHUNKS, BL], BF16, tag="rn_bf16")
        singles_rnbf = [rn_bf16]

        # ---- Phase 1: projections, h1 first. Each m-tile is evacuated
        # straight to its flat DRAM mirror (read back replicated by the
        # bilinear feed DMAs). ----
        with tc.tile_pool(name="xpool", bufs=3) as xpool, tc.tile_pool(
            name="wproj", bufs=2
        ) as wproj, tc.tile_pool(name="hev", bufs=3) as hev_pool:
            for xT, wdram, bias_s, hdram in (
                (x1T, w1, b1s, h1_dram),
                (x0T, w0, b0s, h0_dram),
            ):
                xr = xT.rearrange("(kt p) b -> p kt b", p=128)
                xh = []
                for half in range(2):
                    xt = xpool.tile([128, K_IN // 2, BL], BF16, tag="xs")
                    nc.sync.dma_start(
                        out=xt, in_=xr[:, half * 8 : half * 8 + 8, :]
                    )
                    xh.append(xt)
                for mt in range(MT_H):
                    m0 = mt * 128
                    mw = min(128, MM - m0)
                    wt = wproj.tile([128, K_IN, 128], BF16, tag="wt")
                    nc.sync.dma_start(
                        out=wt.rearrange("p k m -> p (k m)"), in_=wdram[:, mt, :]
                    )
                    hev = hev_pool.tile([128, BL], BF16, tag="hev")
                    for h in range(NH):
                        ps = mm_psum.tile([128, 512], F32, tag="mmps")
                        for kt in range(K_IN):
                            nc.tensor.matmul(
                                out=ps,
                                lhsT=wt[:, kt, :],
                                rhs=xh[kt // 8][:, kt % 8, h * 512 : (h + 1) * 512],
                                start=(kt == 0),
                                stop=(kt == K_IN - 1),
                            )
                        nc.scalar.activation(
                            out=hev[:mw, h * 512 : (h + 1) * 512],
                            in_=ps[:mw, :],
                            func=AF.Identity,
                            bias=bias_s[:mw, mt : mt + 1],
                            scale=1.0,
                        )
                    _ring2(nc).dma_start(out=hdram[m0 : m0 + mw, :], in_=hev[:mw, :])

        # ---- Phases 2+3: bilinear per chunk + norms ----
        with ExitStack() as p23:
            rep_pool = p23.enter_context(tc.tile_pool(name="rep", bufs=2))
            dup_pool = p23.enter_context(tc.tile_pool(name="dup", bufs=2))
            wb_pool = p23.enter_context(tc.tile_pool(name="wbpool", bufs=2))
            p_pool = p23.enter_context(tc.tile_pool(name="ppool", bufs=2))
            post_pool = p23.enter_context(tc.tile_pool(name="post", bufs=2))
            zs_pool = p23.enter_context(tc.tile_pool(name="zs", bufs=6))
            nsq1_pool = p23.enter_context(tc.tile_pool(name="nsq1", bufs=2))
            rnb_pool = p23.enter_context(tc.tile_pool(name="rnb", bufs=2))
            zn_pool = p23.enter_context(tc.tile_pool(name="zn", bufs=2))

            zs_tiles = {}
            for c in range(CHUNKS):
                r0 = CS * c
                # h0 replicated: partition p, slot i <- h0 row 10*(p//16) + i
                # (rows for one partition-group are contiguous, so (i,b)
                # merges and the broadcast AP stays 3-dim)
                h0rep = []
                hsrc = h0_dram[r0 : r0 + CS, :].rearrange("(r i) b -> r (i b)", r=8)
                for ih in range(2):
                    hr = rep_pool.tile([128, 5, BL], BF16, tag="h0rep")
                    src = (
                        hsrc[:, 5 * ih * BL : (5 * ih + 5) * BL]
                        .unsqueeze(1)
                        .broadcast_to([8, 16, 5 * BL])
                    )
                    nc.sync.dma_start(out=hr.rearrange("p i b -> p (i b)"), in_=src)
                    h0rep.append(hr)
                # h1 replicated: partition p, slot j <- h1 row 5*(p%16) + j
                h1dup = dup_pool.tile([128, 5, BL], BF16, tag="h1dup")
                src = (
                    h1_dram[r0 : r0 + CS, :]
                    .rearrange("(u j) b -> u (j b)", u=16)
                    .unsqueeze(0)
                    .broadcast_to([8, 16, 5 * BL])
                )
                _ring2(nc).dma_start(out=h1dup.rearrange("p j b -> p (j b)"), in_=src)
                # all 50 k-tiles of this chunk's weights in one DMA.
                # weight tiles are zero-padded to 128 output columns: FWL
                # (fast weight load) only engages at exactly 128 weight
                # cols, and with it a N=512 bf16 matmul sustains ~132ns
                # vs ~256ns at M=80.
                wbt = wb_pool.tile([128, KT_BIL, 128], BF16, tag="wbt")
                if c < 2:
                    # zero the FWL pad columns once per pool buffer; later
                    # chunks reuse the same bytes and only rewrite [:, :, :CS]
                    nc.vector.memset(wbt, 0.0)
                _ring2(nc).dma_start(out=wbt[:, :, :CS], in_=wbp[c])

                zps = zpsum_pool.tile([128, BL], F32, tag="zps")
                for i in range(10):
                    pt5 = p_pool.tile([128, 5, BL], BF16, tag="pt5")
                    eng = nc.vector
                    eng.tensor_mul(
                        pt5,
                        h0rep[i // 5][:, i % 5, :]
                        .unsqueeze(1)
                        .broadcast_to([128, 5, BL]),
                        h1dup,
                    )
                    # h outer within i so the PSUM bank switches every 5
                    # MMs, not every MM (bank cycling stalls the PE)
                    for h in range(NH):
                        for j in range(5):
                            kt = 5 * i + j
                            nc.tensor.matmul(
                                out=zps[:, h * 512 : (h + 1) * 512],
                                lhsT=wbt[:, kt, :],
                                rhs=pt5[:, j, h * 512 : (h + 1) * 512],
                                start=(kt == 0),
                                stop=(kt == KT_BIL - 1),
                            )
                # post: a = |z+bb|, g = sign(z+bb), s = sqrt(a), zs = s*g
                av = post_pool.tile([CS, BL], BF16, tag="av")
                nc.scalar.activation(
                    out=av, in_=zps[:CS], func=AF.Abs, bias=bbs[:, c : c + 1],
                    scale=1.0,
                )
                gv = post_pool.tile([CS, BL], BF16, tag="gv")
                nc.scalar.activation(
                    out=gv, in_=zps[:CS], func=AF.Sign, bias=bbs[:, c : c + 1],
                    scale=1.0,
                )
                sv = post_pool.tile([CS, BL], BF16, tag="sv")
                nc.scalar.activation(out=sv, in_=av, func=AF.Sqrt)
                zst = zs_pool.tile([CS, BL], BF16, tag="zst")
                nc.gpsimd.tensor_mul(zst, sv, gv)
                zs_tiles[c] = zst
                # nsq[b] = sum_o |z+bb| ( = ||signsqrt(z)||^2 )
                nps = nsq_psum.tile([1, BL], F32, tag="nps")
                for h in range(NH):
                    nc.tensor.matmul(
                        out=nps[:, h * 512 : (h + 1) * 512],
                        lhsT=ones80,
                        rhs=av[:, h * 512 : (h + 1) * 512],
                        start=True,
                        stop=True,
                    )
                nsq1 = nsq1_pool.tile([1, BL], F32, tag="nsq1")
                nc.scalar.copy(nsq1, nps)
                _ring2(nc).dma_start(out=nsq_big[c : c + 1, :], in_=nsq1)

                # norm batch after every HALF chunks
                if c % HALF == HALF - 1:
                    lo, hi = c - HALF + 1, c + 1
                    # ACT/DVE need start-partition 0: recompute the whole
                    # [20, BL] strip each batch (idempotent, tiny)
                    nc.scalar.activation(out=rn_big, in_=nsq_big, func=AF.Sqrt)
                    nc.vector.tensor_scalar_max(rn_big, rn_big, 1e-12)
                    rn_bf = singles_rnbf[0]
                    with nc.allow_low_precision(reason="rn applied to bf16 z"):
                        nc.vector.reciprocal(rn_bf, rn_big)
                    _ring2(nc).dma_start(out=rn_dram[lo:hi, :], in_=rn_bf[lo:hi])
                    for cc in range(lo, hi):
                        rnb = rnb_pool.tile([CS, BL], BF16, tag="rnb")
                        _ring2(nc).dma_start(
                            out=rnb,
                            in_=rn_dram[cc : cc + 1, :].partition_broadcast(CS),
                        )
                        zn = zn_pool.tile([CS, BL], BF16, tag="zn")
                        nc.gpsimd.tensor_mul(zn, zs_tiles.pop(cc), rnb)
                        if CFG["zres"]:
                            for off, p0, kt, n in _h_row_segments(CS * cc, CS):
                                nc.sync.dma_start(
                                    out=zbig[p0 : p0 + n, kt, :],
                                    in_=zn[off : off + n, :],
                                )
                        else:
                            nc.sync.dma_start(
                                out=z_dram[CS * cc : CS * cc + CS, :], in_=zn
                            )

        # ---- Phase 4: out^T = Wout^T-style matmul + bout ----
        ZKG = (5, 5, 3)  # k-tile groups for the z reload
        with tc.tile_pool(name="zk", bufs=1) as zk_pool, tc.tile_pool(
            name="wo", bufs=3
        ) as wo_pool, tc.tile_pool(name="opool", bufs=2) as o_pool:
            zk = []
            if CFG["zres"]:
                for kt in range(MT_H):
                    zk.append((zbig, kt))
            else:
                kt0 = 0
                for gi, gn in enumerate(ZKG):
                    zt = zk_pool.tile([128, gn, BL], BF16, tag=f"zk{gi}")
                    nc.sync.dma_start(
                        out=zt,
                        in_=z_dram[kt0 * 128 : (kt0 + gn) * 128, :].rearrange(
                            "(kt p) b -> p kt b", p=128
                        ),
                    )
                    for k in range(gn):
                        zk.append((zt, k))
                    kt0 += gn
            for mt in range(MT_O):
                m0 = mt * 128
                mw = min(128, D_OUT - m0)
                wot = wo_pool.tile([128, MT_H, 128], BF16, tag="wot")
                nc.sync.dma_start(
                    out=wot.rearrange("p k m -> p (k m)"), in_=wout[:, mt, :]
                )
                ot = o_pool.tile([128, BL], F32, tag="ot")
                for h in range(NH):
                    ps = mm_psum.tile([128, 512], F32, tag="mmps")
                    for kt in range(MT_H):
                        zt, k = zk[kt]
                        nc.tensor.matmul(
                            out=ps,
                            lhsT=wot[:, kt, :],
                            rhs=zt[:, k, h * 512 : (h + 1) * 512],
                            start=(kt == 0),
                            stop=(kt == MT_H - 1),
                        )
                    nc.scalar.activation(
                        out=ot[:mw, h * 512 : (h + 1) * 512],
                        in_=ps[:mw, :],
                        func=AF.Identity,
                        bias=bouts[:mw, mt : mt + 1],
                        scale=1.0,
                    )
                nc.sync.dma_start(out=outT[m0 : m0 + mw, :], in_=ot[:mw, :])


_PROGRAM = None


def _get_program():
    global _PROGRAM
    if _PROGRAM is None:
        _PROGRAM = build_program()
    return _PROGRAM


def prep_weights(W0, b0, W1, b1, Wb, bb, Wout, bout):
    bf = ml_dtypes.bfloat16

    def pack_proj(W, cols_pad):
        # [K, M] -> [p, mt, kt*128+m]: each m-tile's weights contiguous
        # per partition (one 4KB descriptor per partition per load)
        K, Mfull = W.shape
        Wp = np.zeros((K, cols_pad), np.float32)
        Wp[:, :Mfull] = W
        kt_n, mt_n = K // 128, cols_pad // 128
        return np.ascontiguousarray(
            Wp.reshape(kt_n, 128, mt_n, 128).transpose(1, 2, 0, 3)
            .reshape(128, mt_n, kt_n * 128)
        ).astype(bf)

    w0 = pack_proj(np.asarray(W0, np.float32), MM_PAD)
    w1 = pack_proj(np.asarray(W1, np.float32), MM_PAD)
    # wbp[c, p, 5i+j, o] = Wb[c, o, 10*(p//16) + i, 5*(p%16) + j]
    p = np.arange(128)
    wbp = np.empty((CHUNKS, 128, KT_BIL, CS), dtype=bf)
    for i in range(10):
        s_idx = 10 * (p // 16) + i
        for j in range(5):
            t_idx = 5 * (p % 16) + j
            wbp[:, :, 5 * i + j, :] = Wb[:, :, s_idx, t_idx].transpose(0, 2, 1)
    Woutp = np.zeros((MM_PAD, D_OUT_PAD), np.float32)
    Woutp[:MM, :D_OUT] = Wout
    woutp = np.ascontiguousarray(
        Woutp.reshape(MT_H, 128, MT_O, 128).transpose(1, 2, 0, 3)
        .reshape(128, MT_O, MT_H * 128)
    ).astype(bf)
    b0p = np.zeros(MM_PAD, np.float32)
    b0p[:MM] = b0
    b0c = np.ascontiguousarray(b0p.reshape(MT_H, 128).T)
    b1p = np.zeros(MM_PAD, np.float32)
    b1p[:MM] = b1
    b1c = np.ascontiguousarray(b1p.reshape(MT_H, 128).T)
    bbT = np.ascontiguousarray(np.asarray(bb, np.float32).T)
    boutp = np.zeros(D_OUT_PAD, np.float32)
    boutp[:D_OUT] = bout
    boutc = np.ascontiguousarray(boutp.reshape(MT_O, 128).T)
    return dict(
        w0=w0, w1=w1, wbp=wbp, wout=woutp, b0c=b0c, b1c=b1c, bbT=bbT, boutc=boutc
    )


def make_in_maps(x0, x1, weights):
    bf = ml_dtypes.bfloat16
    x0T = np.ascontiguousarray(np.asarray(x0, np.float32).T).astype(bf)
    x1T = np.ascontiguousarray(np.asarray(x1, np.float32).T).astype(bf)
    in_maps = []
    for r in range(N_CORES):
        sl = slice(r * BL, (r + 1) * BL)
        m = dict(weights)
        m["x0T"] = np.ascontiguousarray(x0T[:, sl])
        m["x1T"] = np.ascontiguousarray(x1T[:, sl])
        in_maps.append(m)
    return in_maps


def run(x0, x1, weights, **kwargs):
    nc = _get_program()
    in_maps = make_in_maps(x0, x1, weights)
    res = run_bass_kernel_spmd(nc, in_maps, core_ids=list(range(N_CORES)), **kwargs)
    out = np.empty((B, D_OUT), np.float32)
    for r in range(N_CORES):
        out[r * BL : (r + 1) * BL, :] = res.results[r]["outT"].T
    return out, res


def kernel(x0, x1, W0, b0, W1, b1, Wb, bb, Wout, bout):
    weights = prep_weights(W0, b0, W1, b1, Wb, bb, Wout, bout)
    out, _ = run(x0, x1, weights)
    return out


# ---- timed runner (no NTFF hook in this container: wall-clock the PJRT
# executable with device-resident inputs, minus dispatch overhead) ----

def _make_sharded_callable(nc, in_maps):
    import jax
    import numpy as _np
    from jax.sharding import Mesh, PartitionSpec, NamedSharding
    from jax.experimental.shard_map import shard_map
    from concourse import bass2jax as b2j
    from concourse import mybir as _mybir

    b2j.install_neuronx_cc_hook()
    n_cores = len(in_maps)
    partition_name = nc.partition_id_tensor.name if nc.partition_id_tensor else None
    in_names, out_names, out_avals, zero_outs = [], [], [], []
    for alloc in nc.m.functions[0].allocations:
        if not isinstance(alloc, _mybir.MemoryLocationSet):
            continue
        name = alloc.memorylocations[0].name
        if alloc.kind == "ExternalInput":
            if name != partition_name:
                in_names.append(name)
        elif alloc.kind == "ExternalOutput":
            shape = tuple(alloc.tensor_shape)
            dtype = _mybir.dt.np(alloc.dtype)
            out_names.append(name)
            out_avals.append(jax.core.ShapedArray(shape, dtype))
            zero_outs.append(_np.zeros(shape, dtype))
    n_params = len(in_names)
    in_names_all = list(in_names) + list(out_names)
    if partition_name is not None:
        in_names_all.append(partition_name)

    def _body(*args):
        operands = list(args)
        if partition_name is not None:
            operands.append(b2j.partition_id_tensor())
        outs = b2j._bass_exec_p.bind(
            *operands,
            out_avals=tuple(out_avals),
            in_names=tuple(in_names_all),
            out_names=tuple(out_names),
            lowering_input_output_aliases=(),
            sim_require_finite=True,
            sim_require_nnan=True,
            nc=nc,
        )
        return tuple(outs)

    devices = jax.devices()[:n_cores]
    mesh = Mesh(_np.asarray(devices), ("core",))
    spec = PartitionSpec("core")
    in_specs = (spec,) * (n_params + len(out_names))
    out_specs = (spec,) * len(out_names)
    n_outs = len(out_names)
    donate = tuple(range(n_params, n_params + n_outs))
    sharded = jax.jit(
        shard_map(_body, mesh=mesh, in_specs=in_specs, out_specs=out_specs,
                  check_rep=False),
        keep_unused=True,
        donate_argnums=donate,
    )
    sh = NamedSharding(mesh, spec)
    concat_in = [
        jax.device_put(
            _np.concatenate([_np.asarray(in_maps[c][n]) for c in range(n_cores)], 0), sh
        )
        for n in in_names
    ]
    state = {"outs": None}

    def _fresh_zeros():
        return [
            jax.device_put(_np.zeros((n_cores * z.shape[0], *z.shape[1:]), z.dtype), sh)
            for z in zero_outs
        ]

    def call():
        seeds = state["outs"] if state["outs"] is not None else _fresh_zeros()
        outs = sharded(*concat_in, *seeds)
        state["outs"] = list(outs)
        return outs
    return call, out_names, out_avals



# revision 10
# speedup vs baseline: 1.0676x; 1.0676x over previous
"""BlockTucker fusion kernel for 8 Trainium2 NeuronCores.

Reference computation (per batch row b):
    h0 = x0 @ W0 + b0; h1 = x1 @ W1 + b1              # [B, 1600]
    per chunk c (20 chunks of 80):
        z[c,o] = sum_{s,t} h0c[s] Wb[c,o,s,t] h1c[t] + bb[c,o]
        z = signsqrt(z); z /= max(||z||_2, 1e-12)
    out = concat(z) @ Wout + bout                      # [B, 3000]

Strategy: pure data parallel over batch (1024 rows/core), bf16 compute.
The bilinear form is an outer-product matmul: per chunk,
P^T[(s,t), b] = h0[s,b]*h1[t,b] is built feature-major and
z^T[o,b] = sum_{st} WbT[(s,t),o] P^T[(s,t),b] accumulates in PSUM.

v2 data-movement design (HW ablation showed the v1 kernel was bound on
SBUF->SBUF replication DMAs that hammer 1-2 of the 16 SBUF AXI ports):
  - both h0 and h1 are evacuated to flat DRAM mirrors; the replicated
    feature-major tiles (h0 row s on 16 partitions / h1 row t on 8) are
    built by ONE batched DRAM->SBUF broadcast DMA each per chunk.
  - Wb is host-repacked so each chunk's 50 k-tiles load as a single
    contiguous 8KB-per-partition DMA.
  - z is staged through DRAM instead of a resident SBUF tile, removing
    the per-chunk SBUF->SBUF repack DMAs.
  - h1 projection runs before h0 so the bilinear feed pipeline starts
    during the h0 projection.
"""

import sys

sys.path.insert(0, "/opt/trn_rl_repo")

from contextlib import ExitStack

import numpy as np
import ml_dtypes

import concourse.bass as bass
import concourse.mybir as mybir
import concourse.tile as tile
from concourse import bacc
from concourse.bass_utils import run_bass_kernel_spmd

BF16 = mybir.dt.bfloat16
F32 = mybir.dt.float32
AF = mybir.ActivationFunctionType

B = 8192
D_IN = 2048
MM = 1600
CHUNKS = 20
CS = 80
D_OUT = 3000
N_CORES = 8
BL = B // N_CORES  # 1024 batch rows per core

K_IN = D_IN // 128  # 16 k-tiles for projections
MT_H = 13  # m-tiles for H (1600 -> 12x128 + 64)
MM_PAD = MT_H * 128  # 1664
KT_BIL = 50  # k-tiles per chunk for bilinear (6400/128)
MT_O = 24  # m-tiles for out (3000 -> 23x128 + 56)
D_OUT_PAD = MT_O * 128  # 3072
NH = BL // 512  # 2 free-dim halves of 512
HALF = 4  # norm-batch granularity (chunks)

CFG = dict(
    allsync=True,
    zres=True,
    # ablation flags (timing experiments only; defaults = graded program)
    abl_proj=True,      # phase 1 (projections)
    abl_bil=True,       # phases 2+3 (bilinear + norms)
    abl_out=True,       # phase 4 (out projection)
    abl_pt5=True,       # the DVE outer-product builds
    abl_repdma=True,    # h0rep/h1dup replication DMAs
    abl_wbdma=True,     # wbt weight DMAs
    abl_bilmm=True,     # bilinear matmuls
)


def _h_row_segments(mm0, nrows):
    segs = []
    a = mm0
    while a < mm0 + nrows:
        p0 = a % 128
        kt = a // 128
        n = min(128 - p0, mm0 + nrows - a)
        segs.append((a - mm0, p0, kt, n))
        a += n
    return segs


def build_program(reps=1):
    nc = bacc.Bacc("TRN2", target_bir_lowering=False, debug=False)

    x0T = nc.dram_tensor("x0T", [D_IN, BL], BF16, kind="ExternalInput").ap()
    x1T = nc.dram_tensor("x1T", [D_IN, BL], BF16, kind="ExternalInput").ap()
    w0 = nc.dram_tensor("w0", [128, MT_H, K_IN * 128], BF16, kind="ExternalInput").ap()
    w1 = nc.dram_tensor("w1", [128, MT_H, K_IN * 128], BF16, kind="ExternalInput").ap()
    wbp = nc.dram_tensor(
        "wbp", [CHUNKS, 128, KT_BIL, CS], BF16, kind="ExternalInput"
    ).ap()
    wout = nc.dram_tensor("wout", [128, MT_O, MT_H * 128], BF16, kind="ExternalInput").ap()
    b0c = nc.dram_tensor("b0c", [128, MT_H], F32, kind="ExternalInput").ap()
    b1c = nc.dram_tensor("b1c", [128, MT_H], F32, kind="ExternalInput").ap()
    bbT = nc.dram_tensor("bbT", [CS, CHUNKS], F32, kind="ExternalInput").ap()
    boutc = nc.dram_tensor("boutc", [128, MT_O], F32, kind="ExternalInput").ap()
    outT = nc.dram_tensor("outT", [D_OUT, BL], F32, kind="ExternalOutput").ap()
    h0_dram = nc.dram_tensor("h0_dram", [MM_PAD, BL], BF16).ap()
    h1_dram = nc.dram_tensor("h1_dram", [MM_PAD, BL], BF16).ap()
    z_dram = nc.dram_tensor("z_dram", [MM_PAD, BL], BF16).ap()
    rn_dram = nc.dram_tensor("rn_dram", [CHUNKS, BL], BF16).ap()

    with tile.TileContext(nc) as tc:
        for _ in range(reps):
            _emit(
                tc, nc, x0T, x1T, w0, w1, wbp, wout, b0c, b1c, bbT, boutc,
                outT, h0_dram, h1_dram, z_dram, rn_dram,
            )
    nc.compile()
    return nc


def _ring2(nc):
    return nc.sync if CFG["allsync"] else nc.scalar


def _emit(
    tc, nc, x0T, x1T, w0, w1, wbp, wout, b0c, b1c, bbT, boutc, outT,
    h0_dram, h1_dram, z_dram, rn_dram,
):
    ctx = ExitStack()
    with ctx:
        singles = ctx.enter_context(tc.tile_pool(name="singles", bufs=1))
        mm_psum = ctx.enter_context(tc.tile_pool(name="mm_psum", bufs=2, space="PSUM"))
        zpsum_pool = ctx.enter_context(tc.tile_pool(name="zpsum", bufs=2, space="PSUM"))
        nsq_psum = ctx.enter_context(tc.tile_pool(name="nsq_psum", bufs=1, space="PSUM"))

        # constants / biases
        b0s = singles.tile([128, MT_H], F32)
        nc.sync.dma_start(out=b0s, in_=b0c)
        b1s = singles.tile([128, MT_H], F32)
        nc.sync.dma_start(out=b1s, in_=b1c)
        bbs = singles.tile([CS, CHUNKS], F32)
        nc.sync.dma_start(out=bbs, in_=bbT)
        bouts = singles.tile([128, MT_O], F32)
        nc.sync.dma_start(out=bouts, in_=boutc)
        ones80 = singles.tile([CS, 1], BF16)
        nc.vector.memset(ones80, 1.0)
        # zero the z_dram pad rows so the final matmul's last k-tile can't
        # see stale NaNs (wout pad rows are zero, but 0*NaN=NaN)
        if CFG["zres"]:
            zbig = singles.tile([128, MT_H, BL], BF16, tag="zbig")
            nc.vector.memset(zbig[64:128, MT_H - 1, :], 0.0)
        else:
            zbig = None
            zpad = singles.tile([64, BL], BF16)
            nc.vector.memset(zpad, 0.0)
            nc.sync.dma_start(out=z_dram[MM : MM_PAD, :], in_=zpad)

        nsq_big = singles.tile([CHUNKS, BL], F32)
        nc.vector.memset(nsq_big, 1.0)
        rn_big = singles.tile([CHUNKS, BL], F32)
        rn_bf16 = singles.tile([CHUNKS, BL], BF16, tag="rn_bf16")
        singles_rnbf = [rn_bf16]

        # ---- Phase 1: projections, h1 first. Each m-tile is evacuated
        # straight to its flat DRAM mirror (read back replicated by the
        # bilinear feed DMAs). ----
        with tc.tile_pool(name="xpool", bufs=3) as xpool, tc.tile_pool(
            name="wproj", bufs=2
        ) as wproj, tc.tile_pool(name="hev", bufs=3) as hev_pool:
            for xT, wdram, bias_s, hdram in (
                ((x1T, w1, b1s, h1_dram),
                 (x0T, w0, b0s, h0_dram)) if CFG["abl_proj"] else ()
            ):
                xr = xT.rearrange("(kt p) b -> p kt b", p=128)
                xh = []
                for half in range(2):
                    xt = xpool.tile([128, K_IN // 2, BL], BF16, tag="xs")
                    nc.sync.dma_start(
                        out=xt, in_=xr[:, half * 8 : half * 8 + 8, :]
                    )
                    xh.append(xt)
                for mt in range(MT_H):
                    m0 = mt * 128
                    mw = min(128, MM - m0)
                    wt = wproj.tile([128, K_IN, 128], BF16, tag="wt")
                    nc.sync.dma_start(
                        out=wt.rearrange("p k m -> p (k m)"), in_=wdram[:, mt, :]
                    )
                    hev = hev_pool.tile([128, BL], BF16, tag="hev")
                    for h in range(NH):
                        ps = mm_psum.tile([128, 512], F32, tag="mmps")
                        for kt in range(K_IN):
                            nc.tensor.matmul(
                                out=ps,
                                lhsT=wt[:, kt, :],
                                rhs=xh[kt // 8][:, kt % 8, h * 512 : (h + 1) * 512],
                                start=(kt == 0),
                                stop=(kt == K_IN - 1),
                            )
                        nc.scalar.activation(
                            out=hev[:mw, h * 512 : (h + 1) * 512],
                            in_=ps[:mw, :],
                            func=AF.Identity,
                            bias=bias_s[:mw, mt : mt + 1],
                            scale=1.0,
                        )
                    _ring2(nc).dma_start(out=hdram[m0 : m0 + mw, :], in_=hev[:mw, :])

        # ---- Phases 2+3: bilinear per chunk + norms ----
        with ExitStack() as p23:
            rep_pool = p23.enter_context(tc.tile_pool(name="rep", bufs=2))
            dup_pool = p23.enter_context(tc.tile_pool(name="dup", bufs=2))
            wb_pool = p23.enter_context(tc.tile_pool(name="wbpool", bufs=2))
            p_pool = p23.enter_context(tc.tile_pool(name="ppool", bufs=2))
            post_pool = p23.enter_context(tc.tile_pool(name="post", bufs=2))
            zs_pool = p23.enter_context(tc.tile_pool(name="zs", bufs=6))
            nsq1_pool = p23.enter_context(tc.tile_pool(name="nsq1", bufs=2))
            rnb_pool = p23.enter_context(tc.tile_pool(name="rnb", bufs=2))
            zn_pool = p23.enter_context(tc.tile_pool(name="zn", bufs=2))

            zs_tiles = {}
            abl = {}  # reused tiles for ablated producers
            for c in range(CHUNKS if CFG["abl_bil"] else 0):
                r0 = CS * c
                # h0 replicated: partition p, slot i <- h0 row 10*(p//16) + i
                # (rows for one partition-group are contiguous, so (i,b)
                # merges and the broadcast AP stays 3-dim)
                hsrc = h0_dram[r0 : r0 + CS, :].rearrange("(r i) b -> r (i b)", r=8)
                if CFG["abl_repdma"]:
                    h0rep = []
                    for ih in range(2):
                        hr = rep_pool.tile([128, 5, BL], BF16, tag="h0rep")
                        src = (
                            hsrc[:, 5 * ih * BL : (5 * ih + 5) * BL]
                            .unsqueeze(1)
                            .broadcast_to([8, 16, 5 * BL])
                        )
                        nc.sync.dma_start(
                            out=hr.rearrange("p i b -> p (i b)"), in_=src
                        )
                        h0rep.append(hr)
                    # h1 replicated: partition p, slot j <- h1 row 5*(p%16)+j
                    h1dup = dup_pool.tile([128, 5, BL], BF16, tag="h1dup")
                    src = (
                        h1_dram[r0 : r0 + CS, :]
                        .rearrange("(u j) b -> u (j b)", u=16)
                        .unsqueeze(0)
                        .broadcast_to([8, 16, 5 * BL])
                    )
                    _ring2(nc).dma_start(
                        out=h1dup.rearrange("p j b -> p (j b)"), in_=src
                    )
                else:
                    if "h0rep" not in abl:
                        hrs = []
                        for ih in range(2):
                            hr = rep_pool.tile([128, 5, BL], BF16, tag="h0rep")
                            nc.gpsimd.memset(hr, 0.5)
                            hrs.append(hr)
                        hd = dup_pool.tile([128, 5, BL], BF16, tag="h1dup")
                        nc.gpsimd.memset(hd, 0.5)
                        abl["h0rep"], abl["h1dup"] = hrs, hd
                    h0rep, h1dup = abl["h0rep"], abl["h1dup"]
                # all 50 k-tiles of this chunk's weights in one DMA.
                # weight tiles are zero-padded to 128 output columns: FWL
                # (fast weight load) only engages at exactly 128 weight
                # cols, and with it a N=512 bf16 matmul sustains ~132ns
                # vs ~256ns at M=80.
                if CFG["abl_wbdma"]:
                    wbt = wb_pool.tile([128, KT_BIL, 128], BF16, tag="wbt")
                    if c < 2:
                        # zero the FWL pad columns once per pool buffer; later
                        # chunks reuse the same bytes, only rewrite [:, :, :CS]
                        nc.vector.memset(wbt, 0.0)
                    _ring2(nc).dma_start(out=wbt[:, :, :CS], in_=wbp[c])
                else:
                    if "wbt" not in abl:
                        wbt = wb_pool.tile([128, KT_BIL, 128], BF16, tag="wbt")
                        nc.vector.memset(wbt, 0.0)
                        abl["wbt"] = wbt
                    wbt = abl["wbt"]

                if CFG["abl_bilmm"]:
                    zps = zpsum_pool.tile([128, BL], F32, tag="zps")
                else:
                    if "zps" not in abl:
                        zps = zpsum_pool.tile([128, BL], F32, tag="zps")
                        nc.vector.memset(zps, 0.0)
                        abl["zps"] = zps
                    zps = abl["zps"]
                for i in range(10):
                    if CFG["abl_pt5"]:
                        pt5 = p_pool.tile([128, 5, BL], BF16, tag="pt5")
                        eng = nc.vector
                        eng.tensor_mul(
                            pt5,
                            h0rep[i // 5][:, i % 5, :]
                            .unsqueeze(1)
                            .broadcast_to([128, 5, BL]),
                            h1dup,
                        )
                    else:
                        if "pt5" not in abl:
                            ps = []
                            for _ in range(2):
                                p_ = p_pool.tile([128, 5, BL], BF16, tag="pt5")
                                nc.gpsimd.memset(p_, 0.5)
                                ps.append(p_)
                            abl["pt5"] = ps
                        pt5 = abl["pt5"][i % 2]
                    # h outer within i so the PSUM bank switches every 5
                    # MMs, not every MM (bank cycling stalls the PE)
                    if not CFG["abl_bilmm"]:
                        continue
                    for h in range(NH):
                        for j in range(5):
                            kt = 5 * i + j
                            nc.tensor.matmul(
                                out=zps[:, h * 512 : (h + 1) * 512],
                                lhsT=wbt[:, kt, :],
                                rhs=pt5[:, j, h * 512 : (h + 1) * 512],
                                start=(kt == 0),
                                stop=(kt == KT_BIL - 1),
                            )
                # post: a = |z+bb|, g = sign(z+bb), s = sqrt(a), zs = s*g
                av = post_pool.tile([CS, BL], BF16, tag="av")
                nc.scalar.activation(
                    out=av, in_=zps[:CS], func=AF.Abs, bias=bbs[:, c : c + 1],
                    scale=1.0,
                )
                gv = post_pool.tile([CS, BL], BF16, tag="gv")
                nc.scalar.activation(
                    out=gv, in_=zps[:CS], func=AF.Sign, bias=bbs[:, c : c + 1],
                    scale=1.0,
                )
                sv = post_pool.tile([CS, BL], BF16, tag="sv")
                nc.scalar.activation(out=sv, in_=av, func=AF.Sqrt)
                zst = zs_pool.tile([CS, BL], BF16, tag="zst")
                nc.gpsimd.tensor_mul(zst, sv, gv)
                zs_tiles[c] = zst
                # nsq[b] = sum_o |z+bb| ( = ||signsqrt(z)||^2 )
                nps = nsq_psum.tile([1, BL], F32, tag="nps")
                for h in range(NH):
                    nc.tensor.matmul(
                        out=nps[:, h * 512 : (h + 1) * 512],
                        lhsT=ones80,
                        rhs=av[:, h * 512 : (h + 1) * 512],
                        start=True,
                        stop=True,
                    )
                nsq1 = nsq1_pool.tile([1, BL], F32, tag="nsq1")
                nc.scalar.copy(nsq1, nps)
                _ring2(nc).dma_start(out=nsq_big[c : c + 1, :], in_=nsq1)

                # norm batch after every HALF chunks
                if c % HALF == HALF - 1:
                    lo, hi = c - HALF + 1, c + 1
                    # ACT/DVE need start-partition 0: recompute the whole
                    # [20, BL] strip each batch (idempotent, tiny)
                    nc.scalar.activation(out=rn_big, in_=nsq_big, func=AF.Sqrt)
                    nc.vector.tensor_scalar_max(rn_big, rn_big, 1e-12)
                    rn_bf = singles_rnbf[0]
                    with nc.allow_low_precision(reason="rn applied to bf16 z"):
                        nc.vector.reciprocal(rn_bf, rn_big)
                    _ring2(nc).dma_start(out=rn_dram[lo:hi, :], in_=rn_bf[lo:hi])
                    for cc in range(lo, hi):
                        rnb = rnb_pool.tile([CS, BL], BF16, tag="rnb")
                        _ring2(nc).dma_start(
                            out=rnb,
                            in_=rn_dram[cc : cc + 1, :].partition_broadcast(CS),
                        )
                        zn = zn_pool.tile([CS, BL], BF16, tag="zn")
                        nc.gpsimd.tensor_mul(zn, zs_tiles.pop(cc), rnb)
                        if CFG["zres"]:
                            for off, p0, kt, n in _h_row_segments(CS * cc, CS):
                                nc.sync.dma_start(
                                    out=zbig[p0 : p0 + n, kt, :],
                                    in_=zn[off : off + n, :],
                                )
                        else:
                            nc.sync.dma_start(
                                out=z_dram[CS * cc : CS * cc + CS, :], in_=zn
                            )

        # ---- Phase 4: out^T = Wout^T-style matmul + bout ----
        if not CFG["abl_out"]:
            return
        ZKG = (5, 5, 3)  # k-tile groups for the z reload
        with tc.tile_pool(name="zk", bufs=1) as zk_pool, tc.tile_pool(
            name="wo", bufs=3
        ) as wo_pool, tc.tile_pool(name="opool", bufs=2) as o_pool:
            zk = []
            if CFG["zres"]:
                for kt in range(MT_H):
                    zk.append((zbig, kt))
            else:
                kt0 = 0
                for gi, gn in enumerate(ZKG):
                    zt = zk_pool.tile([128, gn, BL], BF16, tag=f"zk{gi}")
                    nc.sync.dma_start(
                        out=zt,
                        in_=z_dram[kt0 * 128 : (kt0 + gn) * 128, :].rearrange(
                            "(kt p) b -> p kt b", p=128
                        ),
                    )
                    for k in range(gn):
                        zk.append((zt, k))
                    kt0 += gn
            for mt in range(MT_O):
                m0 = mt * 128
                mw = min(128, D_OUT - m0)
                wot = wo_pool.tile([128, MT_H, 128], BF16, tag="wot")
                nc.sync.dma_start(
                    out=wot.rearrange("p k m -> p (k m)"), in_=wout[:, mt, :]
                )
                ot = o_pool.tile([128, BL], F32, tag="ot")
                for h in range(NH):
                    ps = mm_psum.tile([128, 512], F32, tag="mmps")
                    for kt in range(MT_H):
                        zt, k = zk[kt]
                        nc.tensor.matmul(
                            out=ps,
                            lhsT=wot[:, kt, :],
                            rhs=zt[:, k, h * 512 : (h + 1) * 512],
                            start=(kt == 0),
                            stop=(kt == MT_H - 1),
                        )
                    nc.scalar.activation(
                        out=ot[:mw, h * 512 : (h + 1) * 512],
                        in_=ps[:mw, :],
                        func=AF.Identity,
                        bias=bouts[:mw, mt : mt + 1],
                        scale=1.0,
                    )
                nc.sync.dma_start(out=outT[m0 : m0 + mw, :], in_=ot[:mw, :])


_PROGRAM = None


def _get_program():
    global _PROGRAM
    if _PROGRAM is None:
        _PROGRAM = build_program()
    return _PROGRAM


def prep_weights(W0, b0, W1, b1, Wb, bb, Wout, bout):
    bf = ml_dtypes.bfloat16

    def pack_proj(W, cols_pad):
        # [K, M] -> [p, mt, kt*128+m]: each m-tile's weights contiguous
        # per partition (one 4KB descriptor per partition per load)
        K, Mfull = W.shape
        Wp = np.zeros((K, cols_pad), np.float32)
        Wp[:, :Mfull] = W
        kt_n, mt_n = K // 128, cols_pad // 128
        return np.ascontiguousarray(
            Wp.reshape(kt_n, 128, mt_n, 128).transpose(1, 2, 0, 3)
            .reshape(128, mt_n, kt_n * 128)
        ).astype(bf)

    w0 = pack_proj(np.asarray(W0, np.float32), MM_PAD)
    w1 = pack_proj(np.asarray(W1, np.float32), MM_PAD)
    # wbp[c, p, 5i+j, o] = Wb[c, o, 10*(p//16) + i, 5*(p%16) + j]
    p = np.arange(128)
    wbp = np.empty((CHUNKS, 128, KT_BIL, CS), dtype=bf)
    for i in range(10):
        s_idx = 10 * (p // 16) + i
        for j in range(5):
            t_idx = 5 * (p % 16) + j
            wbp[:, :, 5 * i + j, :] = Wb[:, :, s_idx, t_idx].transpose(0, 2, 1)
    Woutp = np.zeros((MM_PAD, D_OUT_PAD), np.float32)
    Woutp[:MM, :D_OUT] = Wout
    woutp = np.ascontiguousarray(
        Woutp.reshape(MT_H, 128, MT_O, 128).transpose(1, 2, 0, 3)
        .reshape(128, MT_O, MT_H * 128)
    ).astype(bf)
    b0p = np.zeros(MM_PAD, np.float32)
    b0p[:MM] = b0
    b0c = np.ascontiguousarray(b0p.reshape(MT_H, 128).T)
    b1p = np.zeros(MM_PAD, np.float32)
    b1p[:MM] = b1
    b1c = np.ascontiguousarray(b1p.reshape(MT_H, 128).T)
    bbT = np.ascontiguousarray(np.asarray(bb, np.float32).T)
    boutp = np.zeros(D_OUT_PAD, np.float32)
    boutp[:D_OUT] = bout
    boutc = np.ascontiguousarray(boutp.reshape(MT_O, 128).T)
    return dict(
        w0=w0, w1=w1, wbp=wbp, wout=woutp, b0c=b0c, b1c=b1c, bbT=bbT, boutc=boutc
    )


def make_in_maps(x0, x1, weights):
    bf = ml_dtypes.bfloat16
    x0T = np.ascontiguousarray(np.asarray(x0, np.float32).T).astype(bf)
    x1T = np.ascontiguousarray(np.asarray(x1, np.float32).T).astype(bf)
    in_maps = []
    for r in range(N_CORES):
        sl = slice(r * BL, (r + 1) * BL)
        m = dict(weights)
        m["x0T"] = np.ascontiguousarray(x0T[:, sl])
        m["x1T"] = np.ascontiguousarray(x1T[:, sl])
        in_maps.append(m)
    return in_maps


def run(x0, x1, weights, **kwargs):
    nc = _get_program()
    in_maps = make_in_maps(x0, x1, weights)
    res = run_bass_kernel_spmd(nc, in_maps, core_ids=list(range(N_CORES)), **kwargs)
    out = np.empty((B, D_OUT), np.float32)
    for r in range(N_CORES):
        out[r * BL : (r + 1) * BL, :] = res.results[r]["outT"].T
    return out, res


def kernel(x0, x1, W0, b0, W1, b1, Wb, bb, Wout, bout):
    weights = prep_weights(W0, b0, W1, b1, Wb, bb, Wout, bout)
    out, _ = run(x0, x1, weights)
    return out


# ---- timed runner (no NTFF hook in this container: wall-clock the PJRT
# executable with device-resident inputs, minus dispatch overhead) ----

def _make_sharded_callable(nc, in_maps):
    import jax
    import numpy as _np
    from jax.sharding import Mesh, PartitionSpec, NamedSharding
    from jax.experimental.shard_map import shard_map
    from concourse import bass2jax as b2j
    from concourse import mybir as _mybir

    b2j.install_neuronx_cc_hook()
    n_cores = len(in_maps)
    partition_name = nc.partition_id_tensor.name if nc.partition_id_tensor else None
    in_names, out_names, out_avals, zero_outs = [], [], [], []
    for alloc in nc.m.functions[0].allocations:
        if not isinstance(alloc, _mybir.MemoryLocationSet):
            continue
        name = alloc.memorylocations[0].name
        if alloc.kind == "ExternalInput":
            if name != partition_name:
                in_names.append(name)
        elif alloc.kind == "ExternalOutput":
            shape = tuple(alloc.tensor_shape)
            dtype = _mybir.dt.np(alloc.dtype)
            out_names.append(name)
            out_avals.append(jax.core.ShapedArray(shape, dtype))
            zero_outs.append(_np.zeros(shape, dtype))
    n_params = len(in_names)
    in_names_all = list(in_names) + list(out_names)
    if partition_name is not None:
        in_names_all.append(partition_name)

    def _body(*args):
        operands = list(args)
        if partition_name is not None:
            operands.append(b2j.partition_id_tensor())
        outs = b2j._bass_exec_p.bind(
            *operands,
            out_avals=tuple(out_avals),
            in_names=tuple(in_names_all),
            out_names=tuple(out_names),
            lowering_input_output_aliases=(),
            sim_require_finite=True,
            sim_require_nnan=True,
            nc=nc,
        )
        return tuple(outs)

    devices = jax.devices()[:n_cores]
    mesh = Mesh(_np.asarray(devices), ("core",))
    spec = PartitionSpec("core")
    in_specs = (spec,) * (n_params + len(out_names))
    out_specs = (spec,) * len(out_names)
    n_outs = len(out_names)
    donate = tuple(range(n_params, n_params + n_outs))
    sharded = jax.jit(
        shard_map(_body, mesh=mesh, in_specs=in_specs, out_specs=out_specs,
                  check_rep=False),
        keep_unused=True,
        donate_argnums=donate,
    )
    sh = NamedSharding(mesh, spec)
    concat_in = [
        jax.device_put(
            _np.concatenate([_np.asarray(in_maps[c][n]) for c in range(n_cores)], 0), sh
        )
        for n in in_names
    ]
    state = {"outs": None}

    def _fresh_zeros():
        return [
            jax.device_put(_np.zeros((n_cores * z.shape[0], *z.shape[1:]), z.dtype), sh)
            for z in zero_outs
        ]

    def call():
        seeds = state["outs"] if state["outs"] is not None else _fresh_zeros()
        outs = sharded(*concat_in, *seeds)
        state["outs"] = list(outs)
        return outs
    return call, out_names, out_avals

